# revision 20
# baseline (speedup 1.0000x reference)
"""Trainium2 Bass kernel for the Memoroid linear-recurrence block.

Math (per batch b):
    a = sigmoid(x @ W_a + b_a)          [T, D]
    bm = x @ W_b                        [T, D]
    h_t = a_t * h_{t-1} + bm_t          (h_{-1} = h0, scan over t)
    y = gelu_tanh(h) @ W_y + x @ W_skip [T, D]
Returns (h, y).

Strategy: data-parallel over batch (8 sequences -> 8 cores), bf16
on-chip/over-the-wire (fp32 PSUM accumulation; tolerance is 2e-2 so
bf16's ~0.3% error is fine). The host pre-transposes x to [D, T] so the
kernel needs NO PE transposes: phase-A matmuls produce [d_h, t] tiles
directly (W tile stationary, xT moving), the DVE tensor_tensor_scan runs
the recurrence along the free (t) dim, and h is stored transposed
([D, T]) with the host transposing back. y is produced in natural [t, d]
orientation (gelu(h)^T / xT column slices as the stationary operand).
sigmoid/gelu both run via the tanh activation table (sigmoid(z) =
0.5 + 0.5*tanh(z/2)) so there are no activation-table reloads.
"""

import sys

for _p in ("/opt/trn_rl_repo",):
    if _p not in sys.path:
        sys.path.insert(0, _p)

from contextlib import ExitStack

import numpy as np
import ml_dtypes

import concourse.bass as bass
import concourse.bacc as bacc
import concourse.mybir as mybir
from concourse import tile
from concourse.bass_utils import run_bass_kernel_spmd

B, T, D = 8, 4096, 1024
P = 128
KT = D // P            # 8 partition tiles along any d-dimension
TC = 512               # time-chunk length (scan tile free dim)
NCHUNK = T // TC       # 8
TSUB = TC // P         # 4  (128-row output subtiles per chunk)
NO = D // 512          # 2  (512-wide output column chunks)

f32 = mybir.dt.float32
bf16 = mybir.dt.bfloat16

_CACHE = {}


def _build():
    nc = bacc.Bacc()

    xt_d = nc.declare_dram_parameter("xt", [D, T], bf16, False)
    h0_d = nc.declare_dram_parameter("h0", [D], f32, False)
    wa_d = nc.declare_dram_parameter("wa", [D, D], bf16, False)
    bah_d = nc.declare_dram_parameter("bah", [D], f32, False)  # b_a / 2
    wb_d = nc.declare_dram_parameter("wb", [D, D], bf16, False)
    wy_d = nc.declare_dram_parameter("wy", [D, D], bf16, False)
    ws_d = nc.declare_dram_parameter("ws", [D, D], bf16, False)
    ht_d = nc.declare_dram_parameter("ht_out", [D, T], bf16, True)
    y_d = nc.declare_dram_parameter("y_out", [T, D], bf16, True)

    AF = mybir.ActivationFunctionType
    ALU = mybir.AluOpType

    with tile.TileContext(nc) as tc, ExitStack() as ctx:
        wpool = ctx.enter_context(tc.tile_pool(name="weights", bufs=1))
        const_pool = ctx.enter_context(tc.tile_pool(name="const", bufs=1))
        xt_pool = ctx.enter_context(tc.tile_pool(name="xt", bufs=3))
        sc_pool = ctx.enter_context(tc.tile_pool(name="scan", bufs=2))
        yst_pool = ctx.enter_context(tc.tile_pool(name="yst", bufs=3))
        ps_ab = ctx.enter_context(tc.tile_pool(name="ab", bufs=6, space="PSUM"))
        ps_y = ctx.enter_context(tc.tile_pool(name="ypsum", bufs=2, space="PSUM"))

        # Two HWDGE rings: sync carries xT loads + consts + wb + h stores,
        # scalar carries wa + wy/ws + y stores. Weight DMAs are k-ordered so
        # chunk 0 can start accumulating k=0 ~2us in (k-outer loop below).
        # chunk-0 xT tile first on the sync ring: PE's first dependency.
        xT0 = xt_pool.tile([P, KT * TC], bf16, tag="xT", name="xT0")
        xT0v = xT0[:].rearrange("p (k t) -> p k t", k=KT)
        for k in range(KT):
            nc.sync.dma_start(xT0v[:, k, :], xt_d[k * P : (k + 1) * P, 0:TC])

        bah_all = const_pool.tile([P, KT], f32, name="bah_all")
        nc.sync.dma_start(bah_all[:], bah_d.rearrange("(j p) -> p j", p=P))
        h0_all = const_pool.tile([P, KT], f32, name="h0_all")
        nc.sync.dma_start(h0_all[:], h0_d.rearrange("(j p) -> p j", p=P))
        bah_sb = [bah_all[:, j : j + 1] for j in range(KT)]
        h0_sb = [h0_all[:, j : j + 1] for j in range(KT)]

        # persistent weights: [128, 1024] bf16 tiles. wa on the scalar ring
        # (streams from t=0), wb on the sync ring behind xT0.
        wa_sb, wb_sb, wy_sb, ws_sb = [], [], [], []
        for k in range(KT):
            t_ = wpool.tile([P, D], bf16, tag=f"wa{k}", name=f"wa{k}")
            nc.scalar.dma_start(t_[:], wa_d[k * P : (k + 1) * P, :])
            wa_sb.append(t_)
        for k in range(KT):
            t_ = wpool.tile([P, D], bf16, tag=f"wb{k}", name=f"wb{k}")
            nc.sync.dma_start(t_[:], wb_d[k * P : (k + 1) * P, :])
            wb_sb.append(t_)

        def load_late_weights():
            for k in range(KT):
                for lst, dram, nm in ((wy_sb, wy_d, "wy"), (ws_sb, ws_d, "ws")):
                    t_ = wpool.tile([P, D], bf16, tag=f"{nm}{k}", name=f"{nm}{k}")
                    nc.scalar.dma_start(t_[:], dram[k * P : (k + 1) * P, :])
                    lst.append(t_)

        hT_prev = [None] * KT
        pend = None  # (xTv, gT list) of the previous chunk

        def mm_a(ps, j, k, xTv, c, which):
            w = wa_sb[k] if which == "a" else wb_sb[k]
            nc.tensor.matmul(
                ps[:],
                w[:, j * P : (j + 1) * P],
                xTv[:, k, :],
                start=(k == 0),
                stop=(k == KT - 1),
            )

        def a_gate(c, j, psA):
            # sigmoid(z + ba) = 0.5 + 0.5*tanh(0.5*z + ba/2)
            aT = sc_pool.tile([P, TC], bf16, tag="aT", bufs=8, name=f"aT{c}_{j}")
            nc.scalar.activation(aT[:], psA[:], AF.Tanh, bias=bah_sb[j], scale=0.5)
            nc.gpsimd.tensor_scalar(aT[:], aT[:], 0.5, 0.5, op0=ALU.mult, op1=ALU.add)
            return aT

        def scan_block(c, j, aT, psB, hT_cur, gT_cur):
            # evacuate b to SBUF first: releases the PSUM bank ~2.5us earlier
            # (ScalarE is nearest PSUM and has slack) and the all-SBUF bf16
            # scan runs at the 2x DVE rate instead of the 1x PSUM-operand rate
            bB = sc_pool.tile([P, TC], bf16, tag="bB", bufs=4, name=f"bB{c}_{j}")
            nc.scalar.copy(bB[:], psB[:])
            hT = sc_pool.tile([P, TC], bf16, tag=f"hT{j}", name=f"hT{c}_{j}")
            init = h0_sb[j] if c == 0 else hT_prev[j][:, TC - 1 : TC]
            nc.vector.tensor_tensor_scan(
                hT[:], aT[:], bB[:], init, op0=ALU.mult, op1=ALU.add
            )
            nc.sync.dma_start(
                ht_d[j * P : (j + 1) * P, c * TC : (c + 1) * TC], hT[:]
            )
            gT = sc_pool.tile([P, TC], bf16, tag=f"gT{j}", name=f"gT{c}_{j}")
            nc.scalar.activation(gT[:], hT[:], AF.Gelu_apprx_tanh)
            hT_cur.append(hT)
            gT_cur.append(gT)

        def act_block(c, j, psA, psB, xTv, hT_cur, gT_cur):
            aT = a_gate(c, j, psA)
            scan_block(c, j, aT, psB, hT_cur, gT_cur)

        for c in range(NCHUNK + 1):
            if c < NCHUNK:
                if c == 0:
                    xTv = xT0v
                else:
                    xT = xt_pool.tile([P, KT * TC], bf16, tag="xT", name=f"xT{c}")
                    xTv = xT[:].rearrange("p (k t) -> p k t", k=KT)
                    for k in range(KT):
                        nc.sync.dma_start(
                            xTv[:, k, :], xt_d[k * P : (k + 1) * P, c * TC : (c + 1) * TC]
                        )

                # --- phase A: a/b matmuls + tanh + scan + gelu, per j ---
                hT_cur, gT_cur = [], []
                if c == 0:
                    # k-outer over the first 6 j's (all ab PSUM slots) so PE
                    # has work as soon as wa[k]+xT0[k] land (weights stream in
                    # k order; j-inner would idle PE until all of wa arrived).
                    # psB slots cascade off tanh[j] freeing psA[j].
                    J0 = 3
                    psA0 = [ps_ab.tile([P, TC], f32, tag="ab", name=f"psA0_{j}") for j in range(J0)]
                    psB0 = [ps_ab.tile([P, TC], f32, tag="ab", name=f"psB0_{j}") for j in range(J0)]
                    for k in range(KT):
                        for j in range(J0):
                            mm_a(psA0[j], j, k, xTv, c, "a")
                        for j in range(J0):
                            mm_a(psB0[j], j, k, xTv, c, "b")
                    for j in range(J0):
                        act_block(c, j, psA0[j], psB0[j], xTv, hT_cur, gT_cur)
                    jrest = range(J0, KT)
                else:
                    jrest = range(KT)
                for j in jrest:
                    psA = ps_ab.tile([P, TC], f32, tag="ab", name=f"psA{c}_{j}")
                    psB = ps_ab.tile([P, TC], f32, tag="ab", name=f"psB{c}_{j}")
                    for k in range(KT):
                        mm_a(psA, j, k, xTv, c, "a")
                    for k in range(KT):
                        mm_a(psB, j, k, xTv, c, "b")
                    act_block(c, j, psA, psB, xTv, hT_cur, gT_cur)

                if c == 0:
                    load_late_weights()

            if c >= 1:
                # --- phase B for chunk c-1: y matmuls + stores ---
                xTv_p, gT_p = pend
                t0p = (c - 1) * TC
                last = c - 1 == NCHUNK - 1
                for ts in range(TSUB):
                    yst = yst_pool.tile([P, D], bf16, tag="yst", name=f"yst{c-1}_{ts}")
                    for o in range(NO):
                        psY = ps_y.tile([P, 512], f32, tag="y", name=f"psY{c-1}_{ts}_{o}")
                        for j in range(KT):
                            nc.tensor.matmul(
                                psY[:],
                                gT_p[j][:, ts * P : (ts + 1) * P],
                                wy_sb[j][:, o * 512 : (o + 1) * 512],
                                start=(j == 0),
                                stop=False,
                            )
                        for k in range(KT):
                            nc.tensor.matmul(
                                psY[:],
                                xTv_p[:, k, ts * P : (ts + 1) * P],
                                ws_sb[k][:, o * 512 : (o + 1) * 512],
                                start=False,
                                stop=(k == KT - 1),
                            )
                        nc.scalar.copy(yst[:, o * 512 : (o + 1) * 512], psY[:])
                        if last:
                            # tail: store each 512-half as soon as it's ready
                            nc.scalar.dma_start(
                                y_d[
                                    t0p + ts * P : t0p + (ts + 1) * P,
                                    o * 512 : (o + 1) * 512,
                                ],
                                yst[:, o * 512 : (o + 1) * 512],
                            )
                    if not last:
                        nc.scalar.dma_start(
                            y_d[t0p + ts * P : t0p + (ts + 1) * P, :], yst[:]
                        )

            if c < NCHUNK:
                pend = (xTv, gT_cur)
                hT_prev = hT_cur

    nc.finalize()
    return nc


def kernel(x, h0, W_a, b_a, W_b, W_y, W_skip):
    if "nc" not in _CACHE:
        _CACHE["nc"] = _build()
    nc = _CACHE["nc"]

    bf = ml_dtypes.bfloat16
    wa = np.ascontiguousarray(np.asarray(W_a)).astype(bf)
    wb = np.ascontiguousarray(np.asarray(W_b)).astype(bf)
    wy = np.ascontiguousarray(np.asarray(W_y)).astype(bf)
    ws = np.ascontiguousarray(np.asarray(W_skip)).astype(bf)
    bah = (np.asarray(b_a, dtype=np.float32) * 0.5).copy()
    xt = np.ascontiguousarray(
        np.asarray(x, dtype=np.float32).transpose(0, 2, 1)
    ).astype(bf)  # [B, D, T]
    h0 = np.asarray(h0, dtype=np.float32)

    in_maps = []
    for b in range(B):
        in_maps.append(
            {
                "xt": xt[b],
                "h0": np.ascontiguousarray(h0[b]),
                "wa": wa,
                "bah": bah,
                "wb": wb,
                "wy": wy,
                "ws": ws,
            }
        )

    res = run_bass_kernel_spmd(nc, in_maps, core_ids=list(range(B)))
    h = np.stack([r["ht_out"].T for r in res.results]).astype(np.float32)
    y = np.stack([r["y_out"] for r in res.results]).astype(np.float32)
    return h, y


# revision 21
# speedup vs baseline: 1.0029x; 1.0029x over previous
"""Trainium2 Bass kernel for the Memoroid linear-recurrence block.

Math (per batch b):
    a = sigmoid(x @ W_a + b_a)          [T, D]
    bm = x @ W_b                        [T, D]
    h_t = a_t * h_{t-1} + bm_t          (h_{-1} = h0, scan over t)
    y = gelu_tanh(h) @ W_y + x @ W_skip [T, D]
Returns (h, y).

Strategy: data-parallel over batch (8 sequences -> 8 cores), bf16
on-chip/over-the-wire (fp32 PSUM accumulation; tolerance is 2e-2 so
bf16's ~0.3% error is fine). The host pre-transposes x to [D, T] so the
kernel needs NO PE transposes: phase-A matmuls produce [d_h, t] tiles
directly (W tile stationary, xT moving), the DVE tensor_tensor_scan runs
the recurrence along the free (t) dim, and h is stored transposed
([D, T]) with the host transposing back. y is produced in natural [t, d]
orientation (gelu(h)^T / xT column slices as the stationary operand).
sigmoid/gelu both run via the tanh activation table (sigmoid(z) =
0.5 + 0.5*tanh(z/2)) so there are no activation-table reloads.
"""

import sys

for _p in ("/opt/trn_rl_repo",):
    if _p not in sys.path:
        sys.path.insert(0, _p)

from contextlib import ExitStack

import numpy as np
import ml_dtypes

import concourse.bass as bass
import concourse.bacc as bacc
import concourse.mybir as mybir
from concourse import tile
from concourse.bass_utils import run_bass_kernel_spmd

B, T, D = 8, 4096, 1024
P = 128
KT = D // P            # 8 partition tiles along any d-dimension
TC = 512               # time-chunk length (scan tile free dim)
NCHUNK = T // TC       # 8
TSUB = TC // P         # 4  (128-row output subtiles per chunk)
NO = D // 512          # 2  (512-wide output column chunks)

f32 = mybir.dt.float32
bf16 = mybir.dt.bfloat16

_CACHE = {}


def _build():
    nc = bacc.Bacc()

    xt_d = nc.declare_dram_parameter("xt", [D, T], bf16, False)
    h0_d = nc.declare_dram_parameter("h0", [D], f32, False)
    wa_d = nc.declare_dram_parameter("wa", [D, D], bf16, False)
    bah_d = nc.declare_dram_parameter("bah", [D], f32, False)  # b_a / 2
    wb_d = nc.declare_dram_parameter("wb", [D, D], bf16, False)
    wy_d = nc.declare_dram_parameter("wy", [D, D], bf16, False)
    ws_d = nc.declare_dram_parameter("ws", [D, D], bf16, False)
    ht_d = nc.declare_dram_parameter("ht_out", [D, T], bf16, True)
    y_d = nc.declare_dram_parameter("y_out", [T, D], bf16, True)

    AF = mybir.ActivationFunctionType
    ALU = mybir.AluOpType

    with tile.TileContext(nc) as tc, ExitStack() as ctx:
        wpool = ctx.enter_context(tc.tile_pool(name="weights", bufs=1))
        const_pool = ctx.enter_context(tc.tile_pool(name="const", bufs=1))
        xt_pool = ctx.enter_context(tc.tile_pool(name="xt", bufs=3))
        sc_pool = ctx.enter_context(tc.tile_pool(name="scan", bufs=2))
        yst_pool = ctx.enter_context(tc.tile_pool(name="yst", bufs=3))
        ps_ab = ctx.enter_context(tc.tile_pool(name="ab", bufs=6, space="PSUM"))
        ps_y = ctx.enter_context(tc.tile_pool(name="ypsum", bufs=2, space="PSUM"))

        # Two HWDGE rings: sync carries xT loads + consts + wb + h stores,
        # scalar carries wa + wy/ws + y stores. Weight DMAs are k-ordered so
        # chunk 0 can start accumulating k=0 ~2us in (k-outer loop below).
        # chunk-0 xT tile first on the sync ring: PE's first dependency.
        xT0 = xt_pool.tile([P, KT * TC], bf16, tag="xT", name="xT0")
        xT0v = xT0[:].rearrange("p (k t) -> p k t", k=KT)
        for k in range(KT):
            nc.sync.dma_start(xT0v[:, k, :], xt_d[k * P : (k + 1) * P, 0:TC])

        bah_all = const_pool.tile([P, KT], f32, name="bah_all")
        nc.sync.dma_start(bah_all[:], bah_d.rearrange("(j p) -> p j", p=P))
        h0_all = const_pool.tile([P, KT], f32, name="h0_all")
        nc.sync.dma_start(h0_all[:], h0_d.rearrange("(j p) -> p j", p=P))
        bah_sb = [bah_all[:, j : j + 1] for j in range(KT)]
        h0_sb = [h0_all[:, j : j + 1] for j in range(KT)]

        # persistent weights: [128, 1024] bf16 tiles. wa on the scalar ring
        # (streams from t=0), wb on the sync ring behind xT0.
        wa_sb, wb_sb, wy_sb, ws_sb = [], [], [], []
        for k in range(KT):
            t_ = wpool.tile([P, D], bf16, tag=f"wa{k}", name=f"wa{k}")
            nc.scalar.dma_start(t_[:], wa_d[k * P : (k + 1) * P, :])
            wa_sb.append(t_)
        for k in range(KT):
            t_ = wpool.tile([P, D], bf16, tag=f"wb{k}", name=f"wb{k}")
            nc.sync.dma_start(t_[:], wb_d[k * P : (k + 1) * P, :])
            wb_sb.append(t_)

        def load_late_weights():
            for k in range(KT):
                for lst, dram, nm in ((wy_sb, wy_d, "wy"), (ws_sb, ws_d, "ws")):
                    t_ = wpool.tile([P, D], bf16, tag=f"{nm}{k}", name=f"{nm}{k}")
                    nc.scalar.dma_start(t_[:], dram[k * P : (k + 1) * P, :])
                    lst.append(t_)

        hT_prev = [None] * KT
        pend = None  # (xTv, gT list) of the previous chunk

        def mm_a(ps, j, k, xTv, c, which):
            w = wa_sb[k] if which == "a" else wb_sb[k]
            nc.tensor.matmul(
                ps[:],
                w[:, j * P : (j + 1) * P],
                xTv[:, k, :],
                start=(k == 0),
                stop=(k == KT - 1),
            )

        def a_gate(c, j, psA):
            # sigmoid(z + ba) = 0.5 + 0.5*tanh(0.5*z + ba/2)
            aT = sc_pool.tile([P, TC], bf16, tag="aT", bufs=8, name=f"aT{c}_{j}")
            nc.scalar.activation(aT[:], psA[:], AF.Tanh, bias=bah_sb[j], scale=0.5)
            nc.gpsimd.tensor_scalar(aT[:], aT[:], 0.5, 0.5, op0=ALU.mult, op1=ALU.add)
            return aT

        def scan_block(c, j, aT, psB, hT_cur, gT_cur):
            hT = sc_pool.tile([P, TC], bf16, tag=f"hT{j}", name=f"hT{c}_{j}")
            init = h0_sb[j] if c == 0 else hT_prev[j][:, TC - 1 : TC]
            nc.vector.tensor_tensor_scan(
                hT[:], aT[:], psB[:], init, op0=ALU.mult, op1=ALU.add
            )
            nc.sync.dma_start(
                ht_d[j * P : (j + 1) * P, c * TC : (c + 1) * TC], hT[:]
            )
            gT = sc_pool.tile([P, TC], bf16, tag=f"gT{j}", name=f"gT{c}_{j}")
            nc.scalar.activation(gT[:], hT[:], AF.Gelu_apprx_tanh)
            hT_cur.append(hT)
            gT_cur.append(gT)

        def act_block(c, j, psA, psB, xTv, hT_cur, gT_cur):
            aT = a_gate(c, j, psA)
            scan_block(c, j, aT, psB, hT_cur, gT_cur)

        for c in range(NCHUNK + 1):
            if c < NCHUNK:
                if c == 0:
                    xTv = xT0v
                else:
                    xT = xt_pool.tile([P, KT * TC], bf16, tag="xT", name=f"xT{c}")
                    xTv = xT[:].rearrange("p (k t) -> p k t", k=KT)
                    for k in range(KT):
                        nc.sync.dma_start(
                            xTv[:, k, :], xt_d[k * P : (k + 1) * P, c * TC : (c + 1) * TC]
                        )

                # --- phase A: a/b matmuls + tanh + scan + gelu, per j ---
                hT_cur, gT_cur = [], []
                if c == 0:
                    # k-outer over the first 6 j's (all ab PSUM slots) so PE
                    # has work as soon as wa[k]+xT0[k] land (weights stream in
                    # k order; j-inner would idle PE until all of wa arrived).
                    # psB slots cascade off tanh[j] freeing psA[j].
                    J0 = 3
                    psA0 = [ps_ab.tile([P, TC], f32, tag="ab", name=f"psA0_{j}") for j in range(J0)]
                    psB0 = [ps_ab.tile([P, TC], f32, tag="ab", name=f"psB0_{j}") for j in range(J0)]
                    for k in range(KT):
                        for j in range(J0):
                            mm_a(psA0[j], j, k, xTv, c, "a")
                        for j in range(J0):
                            mm_a(psB0[j], j, k, xTv, c, "b")
                    for j in range(J0):
                        act_block(c, j, psA0[j], psB0[j], xTv, hT_cur, gT_cur)
                    jrest = range(J0, KT)
                else:
                    jrest = range(KT)
                for j in jrest:
                    psA = ps_ab.tile([P, TC], f32, tag="ab", name=f"psA{c}_{j}")
                    psB = ps_ab.tile([P, TC], f32, tag="ab", name=f"psB{c}_{j}")
                    for k in range(KT):
                        mm_a(psA, j, k, xTv, c, "a")
                    for k in range(KT):
                        mm_a(psB, j, k, xTv, c, "b")
                    act_block(c, j, psA, psB, xTv, hT_cur, gT_cur)

                if c == 0:
                    load_late_weights()

            if c >= 1:
                # --- phase B for chunk c-1: y matmuls + stores ---
                xTv_p, gT_p = pend
                t0p = (c - 1) * TC
                last = c - 1 == NCHUNK - 1
                for ts in range(TSUB):
                    yst = yst_pool.tile([P, D], bf16, tag="yst", name=f"yst{c-1}_{ts}")
                    for o in range(NO):
                        psY = ps_y.tile([P, 512], f32, tag="y", name=f"psY{c-1}_{ts}_{o}")
                        for j in range(KT):
                            nc.tensor.matmul(
                                psY[:],
                                gT_p[j][:, ts * P : (ts + 1) * P],
                                wy_sb[j][:, o * 512 : (o + 1) * 512],
                                start=(j == 0),
                                stop=False,
                            )
                        for k in range(KT):
                            nc.tensor.matmul(
                                psY[:],
                                xTv_p[:, k, ts * P : (ts + 1) * P],
                                ws_sb[k][:, o * 512 : (o + 1) * 512],
                                start=False,
                                stop=(k == KT - 1),
                            )
                        nc.scalar.copy(yst[:, o * 512 : (o + 1) * 512], psY[:])
                        if last:
                            # tail: store each 512-half as soon as it's ready
                            nc.scalar.dma_start(
                                y_d[
                                    t0p + ts * P : t0p + (ts + 1) * P,
                                    o * 512 : (o + 1) * 512,
                                ],
                                yst[:, o * 512 : (o + 1) * 512],
                            )
                    if not last:
                        nc.scalar.dma_start(
                            y_d[t0p + ts * P : t0p + (ts + 1) * P, :], yst[:]
                        )

            if c < NCHUNK:
                pend = (xTv, gT_cur)
                hT_prev = hT_cur

    nc.finalize()
    return nc


def kernel(x, h0, W_a, b_a, W_b, W_y, W_skip):
    if "nc" not in _CACHE:
        _CACHE["nc"] = _build()
    nc = _CACHE["nc"]

    bf = ml_dtypes.bfloat16
    wa = np.ascontiguousarray(np.asarray(W_a)).astype(bf)
    wb = np.ascontiguousarray(np.asarray(W_b)).astype(bf)
    wy = np.ascontiguousarray(np.asarray(W_y)).astype(bf)
    ws = np.ascontiguousarray(np.asarray(W_skip)).astype(bf)
    bah = (np.asarray(b_a, dtype=np.float32) * 0.5).copy()
    xt = np.ascontiguousarray(
        np.asarray(x, dtype=np.float32).transpose(0, 2, 1)
    ).astype(bf)  # [B, D, T]
    h0 = np.asarray(h0, dtype=np.float32)

    in_maps = []
    for b in range(B):
        in_maps.append(
            {
                "xt": xt[b],
                "h0": np.ascontiguousarray(h0[b]),
                "wa": wa,
                "bah": bah,
                "wb": wb,
                "wy": wy,
                "ws": ws,
            }
        )

    res = run_bass_kernel_spmd(nc, in_maps, core_ids=list(range(B)))
    h = np.stack([r["ht_out"].T for r in res.results]).astype(np.float32)
    y = np.stack([r["y_out"] for r in res.results]).astype(np.float32)
    return h, y
